# revision 2
# baseline (speedup 1.0000x reference)
"""Trainium2 Bass kernel for nn_MultiHeadedAttention_6416681140387 (v2).

Two-branch windowed video attention; per-core = one (video, frame) pair.
v2 redesign vs baseline:
  - all-bf16 data path (host-cast x, weights); fp32 PSUM accumulation
  - full softmax (no online rescale): S kept in SBUF, one Exp+accum per
    q-block (scores ~N(0,1) after scaling -> no max subtraction needed)
  - branch0 V^T from batched window-major gather + matmul; branch1 V^T
    directly from x via row-rectangular strided lhsT (m-tiles = per-frame
    96/48-token half-rows, no gather, no padding)
  - 3x3 conv with weight-reuse groups + single Lrelu(bias,alpha) epilogue
  - xv uploaded rotated (own frame first) so q comes from xv[0]; key/value
    token order is a rotation, which softmax-attention is invariant to.
  - P0^T / att0 / att1 spilled to DRAM between phases to fit SBUF
"""

import sys

if "/opt/trn_rl_repo" not in sys.path:
    sys.path.insert(0, "/opt/trn_rl_repo")

import math
from contextlib import ExitStack

import numpy as np

import concourse.bass as bass
import concourse.tile as tile
from concourse import bacc, mybir
from concourse.masks import make_identity

F32 = mybir.dt.float32
BF16 = mybir.dt.bfloat16

T = 4
C = 256
H = W = 96
PIX = H * W
NCORES = 8

SC = [1.0 / math.sqrt(2048.0), 1.0 / math.sqrt(8192.0)]
NQB = [[(0, 128), (128, 128), (256, 128), (384, 128), (512, 64)],
      [(0, 128), (128, 16)]]
# branch1 m-tiles: (frame, row0, nrows) -> 96/48 tokens, never cross frames
MT1 = [(f, r0, nr) for f in range(T) for (r0, nr) in ((0, 8), (8, 4))]

Exp = mybir.ActivationFunctionType.Exp
Identity = mybir.ActivationFunctionType.Identity
Lrelu = mybir.ActivationFunctionType.Lrelu


def _v0_pieces(f):
    """Branch0 V^T sub-pieces of frame f: (ti, off, m, j0)."""
    start, end = 576 * f, 576 * f + 576
    out = []
    for ti in range(18):
        lo, hi = ti * 128, ti * 128 + 128
        a, b = max(lo, start), min(hi, end)
        if a < b:
            out.append((ti, a - lo, b - a, a - start))
    return out


def build(nc):
    xv = nc.dram_tensor("xv", [T, C, PIX], BF16, kind="ExternalInput")
    wqt = nc.dram_tensor("wqt", [C, C], BF16, kind="ExternalInput")
    wkt = nc.dram_tensor("wkt", [C, C], BF16, kind="ExternalInput")
    wvt = nc.dram_tensor("wvt", [C, C], BF16, kind="ExternalInput")
    wot = nc.dram_tensor("wot", [9, C, C], BF16, kind="ExternalInput")
    bq = nc.dram_tensor("bq", [C], F32, kind="ExternalInput")
    bk = nc.dram_tensor("bk", [C], F32, kind="ExternalInput")
    bv = nc.dram_tensor("bv", [C], F32, kind="ExternalInput")
    bo = nc.dram_tensor("bo", [C], F32, kind="ExternalInput")
    out = nc.dram_tensor("out", [C, PIX], F32, kind="ExternalOutput")

    alt = [0]

    def bias_copy_alt(dst, src, bias_ap):
        alt[0] ^= 1
        if alt[0]:
            nc.scalar.activation(out=dst, in_=src, func=Identity,
                                 bias=bias_ap, scale=1.0)
        else:
            nc.vector.tensor_scalar_add(dst, src, bias_ap)

    def copy_alt(dst, src):
        alt[0] ^= 1
        if alt[0]:
            nc.scalar.copy(dst, src)
        else:
            nc.vector.tensor_copy(dst, src)

    rr = [0]

    def copy_rr(dst, src):
        # DVE is ~2x faster than ACT and ~3x faster than GpSimd for these
        # strided copies -> give it half the work.
        rr[0] = (rr[0] + 1) % 4
        if rr[0] in (0, 2):
            nc.vector.tensor_copy(dst, src)
        elif rr[0] == 1:
            nc.scalar.copy(dst, src)
        else:
            nc.gpsimd.tensor_copy(dst, src)

    with tile.TileContext(nc, pool_alloc_mode="queue") as tc, ExitStack() as top:
        persist = top.enter_context(tc.tile_pool(name="persist", bufs=1))
        dramp = top.enter_context(tc.tile_pool(name="dram", bufs=1, space="DRAM"))

        wq_sb, wk_sb, wv_sb = [None, None], [None, None], [None, None]
        for name, dt_, lst in (("wq", wqt, wq_sb), ("wk", wkt, wk_sb),
                               ("wv", wvt, wv_sb)):
            for cb in range(2):
                t = persist.tile([128, C], BF16, name=f"{name}{cb}",
                                 tag=f"{name}{cb}")
                nc.sync.dma_start(out=t, in_=dt_.ap()[cb * 128:(cb + 1) * 128, :])
                lst[cb] = t

        def bias_tile(name, dt_):
            t = persist.tile([128, 2], F32, tag=name)
            nc.sync.dma_start(
                out=t, in_=bass.AP(tensor=dt_.ap().tensor, offset=0,
                                   ap=[[1, 128], [128, 2]]))
            return t

        bq_sb = bias_tile("bq", bq)
        bk_sb = bias_tile("bk", bk)
        bv_sb = bias_tile("bv", bv)
        bo_sb = bias_tile("bo", bo)
        ident = persist.tile([128, 128], BF16, name="ident", tag="ident")
        make_identity(nc, ident)
        zrow = persist.tile([128, 98], BF16, name="zrow", tag="zrow")
        nc.vector.memset(zrow, 0.0)

        p0t_dram = dramp.tile([128, 18 * 576], BF16, name="p0td", tag="p0td")
        att0_dram = dramp.tile([128, 98 * 98], BF16, name="att0d", tag="att0d")
        att1_dram = dramp.tile([128, 98 * 98], BF16, name="att1d", tag="att1d")

        def conv1x1(xt, w_sb, co0, bias_ap, dst):
            """dst[128, PIX] bf16 = (w.T x)[co0:co0+128] + bias."""
            for pb in range(18):
                ps = pools["cps"].tile([128, 512], F32, name="cps", tag="cps")
                for cb in range(2):
                    nc.tensor.matmul(
                        ps, w_sb[cb][:, co0:co0 + 128],
                        xt[cb][:, pb * 512:(pb + 1) * 512],
                        start=(cb == 0), stop=(cb == 1))
                bias_copy_alt(dst[:, pb * 512:(pb + 1) * 512], ps, bias_ap)

        pools = {}

        def load_frame(pool, f, tagpfx):
            xt = []
            for cb in range(2):
                t = pool.tile([128, PIX], BF16, name=f"{tagpfx}{cb}",
                              tag=f"{tagpfx}{cb}")
                nc.sync.dma_start(
                    out=t, in_=xv.ap()[f, cb * 128:(cb + 1) * 128, :])
                xt.append(t)
            return xt

        def zero_borders(a):
            av = a.rearrange("p (h w) -> p h w", h=98)
            nc.scalar.copy(a[:, 0:98], zrow)
            nc.scalar.copy(a[:, 97 * 98:98 * 98], zrow)
            zcol = zrow[:, 0:96].rearrange("p (a c) -> p a c", a=96)
            nc.vector.tensor_copy(av[:, 1:97, 0:1], zcol)
            nc.vector.tensor_copy(av[:, 1:97, 97:98], zcol)
            return av

        # ============ phase A: K convs + scores (both branches) ============
        # P1T outlives S (used through phase D1) -> open first (LIFO ring)
        esPT1 = ExitStack()
        p_PT1 = esPT1.enter_context(tc.tile_pool(name="PT1", bufs=1))
        p1t_t = [p_PT1.tile([nr * 12, 144], BF16, name=f"p1t_{i}",
                            tag=f"p1t_{i}")
                 for i, (f, r0, nr) in enumerate(MT1)]

        esS = ExitStack()
        p_S = esS.enter_context(tc.tile_pool(name="S", bufs=1))
        s0_t = [p_S.tile([128, 2304], BF16, name=f"s0_{i}", tag=f"s0_{i}")
                for i in range(len(NQB[0]))]
        s1_t = [p_S.tile([128, 576], BF16, name=f"s1_{i}", tag=f"s1_{i}")
                for i in range(len(NQB[1]))]

        esA = ExitStack()
        p_x = esA.enter_context(tc.tile_pool(name="ax", bufs=2))
        p_k0 = esA.enter_context(tc.tile_pool(name="k0", bufs=1))
        p_k1 = esA.enter_context(tc.tile_pool(name="k1", bufs=1))
        p_qw = esA.enter_context(tc.tile_pool(name="qw", bufs=1))
        pools["cps"] = esA.enter_context(
            tc.tile_pool(name="cps", bufs=3, space="PSUM"))
        p_sps = esA.enter_context(
            tc.tile_pool(name="sps", bufs=3, space="PSUM"))

        # qw0: [128, 16*576]; qw1: [128, 64*144] (both 9216 cols)
        qw = [p_qw.tile([128, 9216], BF16, name=f"qw{b}", tag=f"qw{b}")
              for b in range(2)]

        k1blk = None
        for f in range(T):
            xt = load_frame(p_x, f, "ax")

            if f == 0:
                # q convs + window gathers (own frame is xv[0]); q tiles
                # share the k0 pool slot (same shape, disjoint lifetime)
                for b in range(2):
                    qb_t = p_k0.tile([128, PIX], BF16, name=f"q{b}",
                                     tag="k0f")
                    conv1x1(xt, wq_sb, b * 128, bq_sb[:, b:b + 1], qb_t)
                    if b == 0:
                        qv = qb_t.rearrange(
                            "p (oh hh ow ww) -> p oh hh ow ww",
                            oh=24, hh=4, ow=24, ww=4)
                        for ci in range(16):
                            wy, wx = divmod(ci, 4)
                            dst = qw[0][:, ci * 576:(ci + 1) * 576].rearrange(
                                "p (a c) -> p a c", a=24)
                            copy_rr(dst, qv[:, :, wy, :, wx])
                    else:
                        qv = qb_t.rearrange(
                            "p (oh hh ow ww) -> p oh hh ow ww",
                            oh=12, hh=8, ow=12, ww=8)
                        qwv = qw[1].rearrange(
                            "p (ci a c) -> p ci a c", ci=64, a=12)
                        for wy in range(8):
                            dst = qwv[:, wy * 8:(wy + 1) * 8]
                            src = qv[:, :, wy, :, :].rearrange(
                                "p oh ow ww -> p ww oh ow")
                            copy_rr(dst, src)

            k0f = p_k0.tile([128, PIX], BF16, name="k0f", tag="k0f")
            conv1x1(xt, wk_sb, 0, bk_sb[:, 0:1], k0f)
            if f % 2 == 0:
                k1blk = p_k1.tile([128, 2 * PIX], BF16, name="k1b", tag="k1b")
            conv1x1(xt, wk_sb, 128, bk_sb[:, 1:2],
                    k1blk[:, (f % 2) * PIX:(f % 2 + 1) * PIX])

            # branch0 scores for this frame
            kv = k0f.rearrange("p (oh hh ow ww) -> p oh hh ow ww",
                               oh=24, hh=4, ow=24, ww=4)
            for nqi, (q0, nqsz) in enumerate(NQB[0]):
                for mc in range(2):
                    ps = p_sps.tile([128, 288], F32, name="sps", tag="sps")
                    for ci in range(16):
                        wy, wx = divmod(ci, 4)
                        rhs = kv[:, mc * 12:(mc + 1) * 12, wy, :, wx]
                        lhsT = qw[0][:, ci * 576 + q0:ci * 576 + q0 + nqsz]
                        nc.tensor.matmul(ps[:nqsz], lhsT, rhs,
                                         start=(ci == 0), stop=(ci == 15))
                    copy_alt(
                        s0_t[nqi][:nqsz, f * 576 + mc * 288:
                                  f * 576 + (mc + 1) * 288], ps[:nqsz])

            # branch1 scores per 2-frame block
            if f % 2 == 1:
                kv1 = k1blk.rearrange(
                    "p (fb oh hh ow ww) -> p fb oh hh ow ww",
                    fb=2, oh=12, hh=8, ow=12, ww=8)
                for nqi, (q0, nqsz) in enumerate(NQB[1]):
                    ps = p_sps.tile([128, 288], F32, name="sps", tag="sps")
                    for ci in range(64):
                        wy, wx = divmod(ci, 8)
                        rhs = kv1[:, :, :, wy, :, wx]
                        lhsT = qw[1][:, ci * 144 + q0:ci * 144 + q0 + nqsz]
                        nc.tensor.matmul(ps[:nqsz], lhsT, rhs,
                                         start=(ci == 0), stop=(ci == 63))
                    copy_alt(
                        s1_t[nqi][:nqsz, (f - 1) * 144:(f + 1) * 144],
                        ps[:nqsz])
        esA.close()

        # ============ phase B: softmax + P^T (both branches) ============
        esB = ExitStack()
        p_P = esB.enter_context(tc.tile_pool(name="P", bufs=1))
        p_stat = esB.enter_context(tc.tile_pool(name="stat", bufs=4))
        p0_t = [p_P.tile([128, 2304], BF16, name=f"p0_{i}", tag=f"p0_{i}")
                for i in range(len(NQB[0]))]
        p1_t = [p_P.tile([128, 576], BF16, name=f"p1_{i}", tag=f"p1_{i}")
                for i in range(len(NQB[1]))]

        for b in range(2):
            st = s0_t if b == 0 else s1_t
            pt = p0_t if b == 0 else p1_t
            for nqi, (q0, nqsz) in enumerate(NQB[b]):
                ls = p_stat.tile([128, 1], F32, name="ls", tag="ls")
                nc.scalar.activation(out=pt[nqi][:nqsz, :],
                                     in_=st[nqi][:nqsz, :], func=Exp,
                                     scale=SC[b], accum_out=ls[:nqsz])
                rs = p_stat.tile([128, 1], F32, name="rs", tag="rs")
                nc.vector.reciprocal(rs[:nqsz], ls[:nqsz])
                nc.vector.tensor_scalar_mul(pt[nqi][:nqsz, :],
                                            pt[nqi][:nqsz, :], rs[:nqsz])

        # transposes: P0 -> DRAM-spilled P0T; P1 -> resident P1T
        with tc.tile_pool(name="p0t", bufs=1) as p_P0T, \
             tc.tile_pool(name="ptps", bufs=3, space="PSUM") as p_ptps:
            for ti in range(18):
                p0t = p_P0T.tile([128, 576], BF16, name=f"p0t{ti % 3}",
                                 tag=f"p0t{ti % 3}", bufs=2)
                for nqi, (q0, nqsz) in enumerate(NQB[0]):
                    tp = p_ptps.tile([128, 128], BF16, name="ptps", tag="ptps")
                    nc.tensor.transpose(
                        tp[:, :nqsz],
                        p0_t[nqi][:nqsz, ti * 128:(ti + 1) * 128],
                        ident[:nqsz, :nqsz])
                    copy_alt(p0t[:, q0:q0 + nqsz], tp[:, :nqsz])
                nc.sync.dma_start(out=p0t_dram[:, ti * 576:(ti + 1) * 576],
                                  in_=p0t)
            for i, (f, r0, nr) in enumerate(MT1):
                mt = nr * 12
                c0 = f * 144 + r0 * 12
                for nqi, (q0, nqsz) in enumerate(NQB[1]):
                    tp = p_ptps.tile([128, 128], BF16, name="ptps", tag="ptps")
                    nc.tensor.transpose(
                        tp[:mt, :nqsz],
                        p1_t[nqi][:nqsz, c0:c0 + mt],
                        ident[:nqsz, :nqsz])
                    copy_alt(p1t_t[i][:mt, q0:q0 + nqsz], tp[:mt, :nqsz])
        esB.close()
        esS.close()

        # ============ phase C0+D0: V0^T build, then PV0 -> att0 ============
        esV0 = ExitStack()
        p_V0 = esV0.enter_context(tc.tile_pool(name="V0", bufs=1))
        v0_t = [p_V0.tile([128, 16 * 128], BF16, name=f"v0_{i}",
                          tag=f"v0_{i}") for i in range(18)]
        with ExitStack() as esC0:
            p_x2 = esC0.enter_context(tc.tile_pool(name="c0x", bufs=2))
            p_xw = esC0.enter_context(tc.tile_pool(name="xw0", bufs=1))
            p_vps = esC0.enter_context(
                tc.tile_pool(name="vps", bufs=4, space="PSUM"))
            for f in range(T):
                # window-major gather from quarter-frame x loads; xw0 is
                # double-buffered so frame f+1's gather overlaps f's matmuls
                xw0 = [p_xw.tile([128, 16 * 576], BF16, name=f"xw0_{cb}",
                                 tag=f"xw0_{cb}") for cb in range(2)]
                for q in range(4):
                    for cb in range(2):
                        xq = p_x2.tile([128, 2304], BF16, name=f"c0x{cb}",
                                       tag=f"c0x{cb}")
                        nc.sync.dma_start(
                            out=xq, in_=xv.ap()[f, cb * 128:(cb + 1) * 128,
                                                q * 2304:(q + 1) * 2304])
                        xqv = xq.rearrange(
                            "p (oh hh ow ww) -> p oh hh ow ww",
                            oh=6, hh=4, ow=24, ww=4)
                        for ci in range(16):
                            wy, wx = divmod(ci, 4)
                            dst = xw0[cb][:, ci * 576 + q * 144:
                                          ci * 576 + (q + 1) * 144].rearrange(
                                "p (a c) -> p a c", a=6)
                            copy_rr(dst, xqv[:, :, wy, :, wx])
                for (ti, off, m, j0) in _v0_pieces(f):
                    for cig in range(4):
                        ps = p_vps.tile([128, 512], F32, name="vps", tag="vps")
                        for cii in range(4):
                            ci = cig * 4 + cii
                            for cb in range(2):
                                nc.tensor.matmul(
                                    ps[off:off + m, cii * 128:(cii + 1) * 128],
                                    xw0[cb][:, ci * 576 + j0:ci * 576 + j0 + m],
                                    wv_sb[cb][:, 0:128],
                                    start=(cb == 0), stop=(cb == 1),
                                    tile_position=(0, off))
                        copy_alt(
                            v0_t[ti][off:off + m, cig * 512:(cig + 1) * 512],
                            ps[off:off + m, :])

        # D0: PV0 -> att0 -> DRAM
        with tc.tile_pool(name="att0", bufs=1) as p_att0, \
             tc.tile_pool(name="p0tr", bufs=1) as p_P0Tr, \
             tc.tile_pool(name="pvps", bufs=3, space="PSUM") as p_pvps:
            att0 = p_att0.tile([128, 98 * 98], BF16, name="att0", tag="att0")
            att0v = zero_borders(att0)
            wvw0 = att0v[:, 1:97, 1:97].rearrange(
                "p (oh hh) (ow ww) -> p oh hh ow ww", hh=4, ww=4)
            p0t_r = [p_P0Tr.tile([128, 576], BF16, name=f"p0tr{i}",
                                 tag=f"p0tr{i}") for i in range(18)]
            for ti in range(18):
                nc.sync.dma_start(out=p0t_r[ti],
                                  in_=p0t_dram[:, ti * 576:(ti + 1) * 576])
            for ci in range(16):
                wy, wx = divmod(ci, 4)
                for nqh in range(2):
                    ps = p_pvps.tile([128, 288], F32, name="pvps", tag="pvps")
                    for ti in range(18):
                        nc.tensor.matmul(
                            ps, v0_t[ti][:, ci * 128:(ci + 1) * 128],
                            p0t_r[ti][:, nqh * 288:(nqh + 1) * 288],
                            start=(ti == 0), stop=(ti == 17))
                    dst = wvw0[:, nqh * 12:(nqh + 1) * 12, wy, :, wx]
                    bias_copy_alt(dst, ps.rearrange("p (a c) -> p a c", a=12),
                                  bv_sb[:, 0:1])
            nc.sync.dma_start(out=att0_dram, in_=att0)
        esV0.close()

        # ==== phase C1+D1: V1^T via per-frame window gather, PV1 -> att1 ====
        esV1 = ExitStack()
        p_V1 = esV1.enter_context(tc.tile_pool(name="V1", bufs=1))
        v1_t = [p_V1.tile([nr * 12, 64 * 128], BF16,
                          name=f"v1_{i}", tag=f"v1_{i}")
                for i, (f, r0, nr) in enumerate(MT1)]
        with ExitStack() as esC1:
            p_xw1 = esC1.enter_context(tc.tile_pool(name="xw1", bufs=1))
            p_x3 = esC1.enter_context(tc.tile_pool(name="c1x", bufs=2))
            p_vps1 = esC1.enter_context(
                tc.tile_pool(name="vps1", bufs=4, space="PSUM"))
            xw1f = [p_xw1.tile([128, 64 * 144], BF16, name=f"xw1_{cb}",
                               tag=f"xw1_{cb}") for cb in range(2)]
            for f in range(T):
                for q in range(4):
                    for cb in range(2):
                        xq = p_x3.tile([128, 2304], BF16, name=f"c1x{cb}",
                                       tag=f"c1x{cb}")
                        nc.sync.dma_start(
                            out=xq, in_=xv.ap()[f, cb * 128:(cb + 1) * 128,
                                                q * 2304:(q + 1) * 2304])
                        xqv = xq.rearrange(
                            "p (oh hh ow ww) -> p oh hh ow ww",
                            oh=3, hh=8, ow=12, ww=8)
                        xwv = xw1f[cb].rearrange(
                            "p (ci a c) -> p ci a c", ci=64, a=12)
                        for wy in range(8):
                            dst = xwv[:, wy * 8:(wy + 1) * 8,
                                      q * 3:(q + 1) * 3, :]
                            src = xqv[:, :, wy, :, :].rearrange(
                                "p oh ow ww -> p ww oh ow")
                            copy_rr(dst, src)
                for i, (ff, r0, nr) in enumerate(MT1):
                    if ff != f:
                        continue
                    mt = nr * 12
                    for cig in range(16):
                        ps = p_vps1.tile([128, 512], F32,
                                         name="vps1", tag="vps1")
                        for cii in range(4):
                            ci = cig * 4 + cii
                            for cb in range(2):
                                nc.tensor.matmul(
                                    ps[0:mt, cii * 128:(cii + 1) * 128],
                                    xw1f[cb][:, ci * 144 + r0 * 12:
                                             ci * 144 + r0 * 12 + mt],
                                    wv_sb[cb][:, 128:256],
                                    start=(cb == 0), stop=(cb == 1))
                        copy_alt(
                            v1_t[i][0:mt, cig * 512:(cig + 1) * 512],
                            ps[0:mt, :])

        # D1: PV1 -> att1 (stays resident through E)
        esAtt1 = ExitStack()
        p_att1 = esAtt1.enter_context(tc.tile_pool(name="att1", bufs=1))
        att1 = p_att1.tile([128, 98 * 98], BF16, name="att1", tag="att1")
        att1v = zero_borders(att1)
        wvw1 = att1v[:, 1:97, 1:97].rearrange(
            "p (oh hh) (ow ww) -> p oh hh ow ww", hh=8, ww=8)
        with tc.tile_pool(name="pvps1", bufs=3, space="PSUM") as p_pvps1:
            for ci in range(64):
                wy, wx = divmod(ci, 8)
                ps = p_pvps1.tile([128, 144], F32, name="pvps1", tag="pvps1")
                for i, (f, r0, nr) in enumerate(MT1):
                    mt = nr * 12
                    nc.tensor.matmul(
                        ps, v1_t[i][:mt, ci * 128:(ci + 1) * 128],
                        p1t_t[i][:mt, :],
                        start=(i == 0), stop=(i == len(MT1) - 1))
                dst = wvw1[:, :, wy, :, wx]
                bias_copy_alt(dst, ps.rearrange("p (a c) -> p a c", a=12),
                              bv_sb[:, 1:2])

        # ============ phase E: 3x3 conv + LeakyReLU ============
        # att1 stays resident (esAtt1/esV1/esPT1 close at the very end);
        # att0 is reloaded from its DRAM spill.
        with tc.tile_pool(name="attE", bufs=1) as p_attE, \
             tc.tile_pool(name="wot", bufs=1) as p_wot, \
             tc.tile_pool(name="eout", bufs=2) as p_eo, \
             tc.tile_pool(name="dps", bufs=8, space="PSUM") as p_dps:
            att0E = p_attE.tile([128, 98 * 98], BF16, name="attE0",
                                tag="attE0")
            nc.sync.dma_start(out=att0E, in_=att0_dram)
            att = [att0E, att1]
            wot_sb = []
            for cb in range(2):
                t = p_wot.tile([128, 9, C], BF16, name=f"wot{cb}",
                               tag=f"wot{cb}")
                nc.sync.dma_start(
                    out=t,
                    in_=wot.ap()[:, cb * 128:(cb + 1) * 128, :].rearrange(
                        "t i o -> i t o"))
                wot_sb.append(t)
            attv2 = [att[cb].rearrange("p (h w) -> p h w", h=98)
                     for cb in range(2)]
            for coutb in range(2):
                for g in range(4):          # groups of 6 rg (24 rows each)
                    pst = [p_dps.tile([128, 384], F32, name="dps",
                                      tag="dps") for _ in range(6)]
                    ot = p_eo.tile([128, 6 * 384], F32, name="eo", tag="eo")
                    k = 0
                    for cb in range(2):
                        for tap in range(9):
                            dy, dx = divmod(tap, 3)
                            lhsT = wot_sb[cb][:, tap,
                                              coutb * 128:(coutb + 1) * 128]
                            for ri in range(6):
                                rg = g * 6 + ri
                                rhs = attv2[cb][:, rg * 4 + dy:rg * 4 + dy + 4,
                                                dx:dx + 96]
                                nc.tensor.matmul(pst[ri], lhsT, rhs,
                                                 start=(k == 0), stop=(k == 17))
                            k += 1
                    for ri in range(6):
                        t1 = p_eo.tile([128, 384], F32, name="t1", tag="t1",
                                       bufs=3)
                        nc.scalar.activation(
                            out=t1, in_=pst[ri], func=Identity,
                            bias=bo_sb[:, coutb:coutb + 1], scale=1.0)
                        nc.vector.scalar_tensor_tensor(
                            out=ot[:, ri * 384:(ri + 1) * 384], in0=t1,
                            scalar=0.2, in1=t1,
                            op0=mybir.AluOpType.mult,
                            op1=mybir.AluOpType.max)
                    nc.sync.dma_start(
                        out=out.ap()[coutb * 128:(coutb + 1) * 128,
                                     g * 2304:(g + 1) * 2304],
                        in_=ot)
        esAtt1.close()
        esV1.close()
        esPT1.close()
    return nc


_CACHED = {}


def _get_nc():
    if "nc" not in _CACHED:
        nc = bacc.Bacc("TRN2", debug=False, target_bir_lowering=False)
        build(nc)
        nc.compile()
        _CACHED["nc"] = nc
    return _CACHED["nc"]


def make_in_maps(x, wq, bq_, wk, bk_, wv, bv_, wo, bo_):
    import ml_dtypes
    bf16 = ml_dtypes.bfloat16
    shared = {
        "wqt": np.ascontiguousarray(wq.T).astype(bf16),
        "wkt": np.ascontiguousarray(wk.T).astype(bf16),
        "wvt": np.ascontiguousarray(wv.T).astype(bf16),
        "wot": np.ascontiguousarray(
            wo.transpose(2, 3, 1, 0).reshape(9, C, C)).astype(bf16),
        "bq": np.ascontiguousarray(bq_.astype(np.float32)),
        "bk": np.ascontiguousarray(bk_.astype(np.float32)),
        "bv": np.ascontiguousarray(bv_.astype(np.float32)),
        "bo": np.ascontiguousarray(bo_.astype(np.float32)),
    }
    x3 = x.reshape(2 * T, C, PIX).astype(bf16)
    in_maps = []
    for core in range(NCORES):
        v, f = divmod(core, T)
        idx = [v * T + (f + i) % T for i in range(T)]
        m = dict(shared)
        m["xv"] = np.ascontiguousarray(x3[idx])
        in_maps.append(m)
    return in_maps


def kernel(**inputs):
    from concourse.bass_utils import run_bass_kernel_spmd

    x = np.asarray(inputs["x"], dtype=np.float32)
    in_maps = make_in_maps(
        x, np.asarray(inputs["wq"]), np.asarray(inputs["bq"]),
        np.asarray(inputs["wk"]), np.asarray(inputs["bk"]),
        np.asarray(inputs["wv"]), np.asarray(inputs["bv"]),
        np.asarray(inputs["wo"]), np.asarray(inputs["bo"]))
    nc = _get_nc()
    res = run_bass_kernel_spmd(nc, in_maps, core_ids=list(range(NCORES)))
    outs = [res.results[c]["out"].reshape(C, H, W) for c in range(NCORES)]
    return np.stack(outs).astype(np.float32)


# revision 4
# speedup vs baseline: 1.0166x; 1.0166x over previous
"""Trainium2 Bass kernel for nn_MultiHeadedAttention_6416681140387 (v2).

Two-branch windowed video attention; per-core = one (video, frame) pair.
v2 redesign vs baseline:
  - all-bf16 data path (host-cast x, weights); fp32 PSUM accumulation
  - full softmax (no online rescale): S kept in SBUF, one Exp+accum per
    q-block (scores ~N(0,1) after scaling -> no max subtraction needed)
  - branch0 V^T from batched window-major gather + matmul; branch1 V^T
    directly from x via row-rectangular strided lhsT (m-tiles = per-frame
    96/48-token half-rows, no gather, no padding)
  - 3x3 conv with weight-reuse groups + single Lrelu(bias,alpha) epilogue
  - xv uploaded rotated (own frame first) so q comes from xv[0]; key/value
    token order is a rotation, which softmax-attention is invariant to.
  - P0^T / att0 / att1 spilled to DRAM between phases to fit SBUF
"""

import sys

if "/opt/trn_rl_repo" not in sys.path:
    sys.path.insert(0, "/opt/trn_rl_repo")

import math
from contextlib import ExitStack

import numpy as np

import concourse.bass as bass
import concourse.tile as tile
from concourse import bacc, mybir
from concourse.masks import make_identity

F32 = mybir.dt.float32
BF16 = mybir.dt.bfloat16

T = 4
C = 256
H = W = 96
PIX = H * W
NCORES = 8

SC = [1.0 / math.sqrt(2048.0), 1.0 / math.sqrt(8192.0)]
NQB = [[(0, 128), (128, 128), (256, 128), (384, 128), (512, 64)],
      [(0, 128), (128, 16)]]
# branch1 m-tiles: (frame, row0, nrows) -> 96/48 tokens, never cross frames
MT1 = [(f, r0, nr) for f in range(T) for (r0, nr) in ((0, 8), (8, 4))]

Exp = mybir.ActivationFunctionType.Exp
Identity = mybir.ActivationFunctionType.Identity
Lrelu = mybir.ActivationFunctionType.Lrelu


def _v0_pieces(f):
    """Branch0 V^T sub-pieces of frame f: (ti, off, m, j0)."""
    start, end = 576 * f, 576 * f + 576
    out = []
    for ti in range(18):
        lo, hi = ti * 128, ti * 128 + 128
        a, b = max(lo, start), min(hi, end)
        if a < b:
            out.append((ti, a - lo, b - a, a - start))
    return out


def build(nc):
    xv = nc.dram_tensor("xv", [T, C, PIX], BF16, kind="ExternalInput")
    wqt = nc.dram_tensor("wqt", [C, C], BF16, kind="ExternalInput")
    wkt = nc.dram_tensor("wkt", [C, C], BF16, kind="ExternalInput")
    wvt = nc.dram_tensor("wvt", [C, C], BF16, kind="ExternalInput")
    wot = nc.dram_tensor("wot", [9, C, C], BF16, kind="ExternalInput")
    bq = nc.dram_tensor("bq", [C], F32, kind="ExternalInput")
    bk = nc.dram_tensor("bk", [C], F32, kind="ExternalInput")
    bv = nc.dram_tensor("bv", [C], F32, kind="ExternalInput")
    bo = nc.dram_tensor("bo", [C], F32, kind="ExternalInput")
    out = nc.dram_tensor("out", [C, PIX], F32, kind="ExternalOutput")

    alt = [0]

    def bias_copy_alt(dst, src, bias_ap):
        alt[0] ^= 1
        if alt[0]:
            nc.scalar.activation(out=dst, in_=src, func=Identity,
                                 bias=bias_ap, scale=1.0)
        else:
            nc.vector.tensor_scalar_add(dst, src, bias_ap)

    def copy_alt(dst, src):
        alt[0] ^= 1
        if alt[0]:
            nc.scalar.copy(dst, src)
        else:
            nc.vector.tensor_copy(dst, src)

    rr = [0]

    def copy_rr(dst, src):
        # DVE is ~2x faster than ACT and ~3x faster than GpSimd for these
        # strided copies -> give it half the work.
        rr[0] = (rr[0] + 1) % 4
        if rr[0] in (0, 2):
            nc.vector.tensor_copy(dst, src)
        elif rr[0] == 1:
            nc.scalar.copy(dst, src)
        else:
            nc.gpsimd.tensor_copy(dst, src)

    with tile.TileContext(nc, pool_alloc_mode="queue") as tc, ExitStack() as top:
        persist = top.enter_context(tc.tile_pool(name="persist", bufs=1))
        dramp = top.enter_context(tc.tile_pool(name="dram", bufs=1, space="DRAM"))

        wq_sb, wk_sb, wv_sb = [None, None], [None, None], [None, None]
        for name, dt_, lst in (("wq", wqt, wq_sb), ("wk", wkt, wk_sb),
                               ("wv", wvt, wv_sb)):
            for cb in range(2):
                t = persist.tile([128, C], BF16, name=f"{name}{cb}",
                                 tag=f"{name}{cb}")
                nc.sync.dma_start(out=t, in_=dt_.ap()[cb * 128:(cb + 1) * 128, :])
                lst[cb] = t

        def bias_tile(name, dt_):
            t = persist.tile([128, 2], F32, tag=name)
            nc.sync.dma_start(
                out=t, in_=bass.AP(tensor=dt_.ap().tensor, offset=0,
                                   ap=[[1, 128], [128, 2]]))
            return t

        bq_sb = bias_tile("bq", bq)
        bk_sb = bias_tile("bk", bk)
        bv_sb = bias_tile("bv", bv)
        bo_sb = bias_tile("bo", bo)
        ident = persist.tile([128, 128], BF16, name="ident", tag="ident")
        make_identity(nc, ident)
        zrow = persist.tile([128, 98], BF16, name="zrow", tag="zrow")
        nc.vector.memset(zrow, 0.0)

        p0t_dram = dramp.tile([128, 18 * 576], BF16, name="p0td", tag="p0td")
        att0_dram = dramp.tile([128, 98 * 98], BF16, name="att0d", tag="att0d")
        att1_dram = dramp.tile([128, 98 * 98], BF16, name="att1d", tag="att1d")

        def conv1x1(xt, w_sb, co0, bias_ap, dst):
            """dst[128, PIX] bf16 = (w.T x)[co0:co0+128] + bias."""
            for pb in range(18):
                ps = pools["cps"].tile([128, 512], F32, name="cps", tag="cps")
                for cb in range(2):
                    nc.tensor.matmul(
                        ps, w_sb[cb][:, co0:co0 + 128],
                        xt[cb][:, pb * 512:(pb + 1) * 512],
                        start=(cb == 0), stop=(cb == 1))
                bias_copy_alt(dst[:, pb * 512:(pb + 1) * 512], ps, bias_ap)

        pools = {}

        def load_frame(pool, f, tagpfx):
            xt = []
            for cb in range(2):
                t = pool.tile([128, PIX], BF16, name=f"{tagpfx}{cb}",
                              tag=f"{tagpfx}{cb}")
                nc.sync.dma_start(
                    out=t, in_=xv.ap()[f, cb * 128:(cb + 1) * 128, :])
                xt.append(t)
            return xt

        def zero_borders(a):
            av = a.rearrange("p (h w) -> p h w", h=98)
            nc.scalar.copy(a[:, 0:98], zrow)
            nc.scalar.copy(a[:, 97 * 98:98 * 98], zrow)
            zcol = zrow[:, 0:96].rearrange("p (a c) -> p a c", a=96)
            nc.vector.tensor_copy(av[:, 1:97, 0:1], zcol)
            nc.vector.tensor_copy(av[:, 1:97, 97:98], zcol)
            return av

        # ============ phase A: K convs + scores (both branches) ============
        # P1T outlives S (used through phase D1) -> open first (LIFO ring)
        esPT1 = ExitStack()
        p_PT1 = esPT1.enter_context(tc.tile_pool(name="PT1", bufs=1))
        p1t_t = [p_PT1.tile([nr * 12, 144], BF16, name=f"p1t_{i}",
                            tag=f"p1t_{i}")
                 for i, (f, r0, nr) in enumerate(MT1)]

        esS = ExitStack()
        p_S = esS.enter_context(tc.tile_pool(name="S", bufs=1))
        s0_t = [p_S.tile([128, 2304], BF16, name=f"s0_{i}", tag=f"s0_{i}")
                for i in range(len(NQB[0]))]
        s1_t = [p_S.tile([128, 576], BF16, name=f"s1_{i}", tag=f"s1_{i}")
                for i in range(len(NQB[1]))]

        esA = ExitStack()
        p_x = esA.enter_context(tc.tile_pool(name="ax", bufs=2))
        p_k0 = esA.enter_context(tc.tile_pool(name="k0", bufs=1))
        p_k1 = esA.enter_context(tc.tile_pool(name="k1", bufs=1))
        p_qw = esA.enter_context(tc.tile_pool(name="qw", bufs=1))
        pools["cps"] = esA.enter_context(
            tc.tile_pool(name="cps", bufs=4, space="PSUM"))
        p_sps = esA.enter_context(
            tc.tile_pool(name="sps", bufs=4, space="PSUM"))

        # qw0: [128, 16*576]; qw1: [128, 64*144] (both 9216 cols)
        qw = [p_qw.tile([128, 9216], BF16, name=f"qw{b}", tag=f"qw{b}")
              for b in range(2)]

        k1blk = None
        for f in range(T):
            xt = load_frame(p_x, f, "ax")

            if f == 0:
                # q convs + window gathers (own frame is xv[0]); q tiles
                # share the k0 pool slot (same shape, disjoint lifetime)
                for b in range(2):
                    qb_t = p_k0.tile([128, PIX], BF16, name=f"q{b}",
                                     tag="k0f")
                    conv1x1(xt, wq_sb, b * 128, bq_sb[:, b:b + 1], qb_t)
                    if b == 0:
                        qv = qb_t.rearrange(
                            "p (oh hh ow ww) -> p oh hh ow ww",
                            oh=24, hh=4, ow=24, ww=4)
                        for ci in range(16):
                            wy, wx = divmod(ci, 4)
                            dst = qw[0][:, ci * 576:(ci + 1) * 576].rearrange(
                                "p (a c) -> p a c", a=24)
                            copy_rr(dst, qv[:, :, wy, :, wx])
                    else:
                        qv = qb_t.rearrange(
                            "p (oh hh ow ww) -> p oh hh ow ww",
                            oh=12, hh=8, ow=12, ww=8)
                        qwv = qw[1].rearrange(
                            "p (ci a c) -> p ci a c", ci=64, a=12)
                        for wy in range(8):
                            dst = qwv[:, wy * 8:(wy + 1) * 8]
                            src = qv[:, :, wy, :, :].rearrange(
                                "p oh ow ww -> p ww oh ow")
                            copy_rr(dst, src)

            k0f = p_k0.tile([128, PIX], BF16, name="k0f", tag="k0f")
            conv1x1(xt, wk_sb, 0, bk_sb[:, 0:1], k0f)
            if f % 2 == 0:
                k1blk = p_k1.tile([128, 2 * PIX], BF16, name="k1b", tag="k1b")
            conv1x1(xt, wk_sb, 128, bk_sb[:, 1:2],
                    k1blk[:, (f % 2) * PIX:(f % 2 + 1) * PIX])

            # branch0 scores for this frame
            kv = k0f.rearrange("p (oh hh ow ww) -> p oh hh ow ww",
                               oh=24, hh=4, ow=24, ww=4)
            for nqi, (q0, nqsz) in enumerate(NQB[0]):
                for mc in range(2):
                    ps = p_sps.tile([128, 288], F32, name="sps", tag="sps")
                    for ci in range(16):
                        wy, wx = divmod(ci, 4)
                        rhs = kv[:, mc * 12:(mc + 1) * 12, wy, :, wx]
                        lhsT = qw[0][:, ci * 576 + q0:ci * 576 + q0 + nqsz]
                        nc.tensor.matmul(ps[:nqsz], lhsT, rhs,
                                         start=(ci == 0), stop=(ci == 15))
                    copy_alt(
                        s0_t[nqi][:nqsz, f * 576 + mc * 288:
                                  f * 576 + (mc + 1) * 288], ps[:nqsz])

            # branch1 scores per 2-frame block
            if f % 2 == 1:
                kv1 = k1blk.rearrange(
                    "p (fb oh hh ow ww) -> p fb oh hh ow ww",
                    fb=2, oh=12, hh=8, ow=12, ww=8)
                for nqi, (q0, nqsz) in enumerate(NQB[1]):
                    ps = p_sps.tile([128, 288], F32, name="sps", tag="sps")
                    for ci in range(64):
                        wy, wx = divmod(ci, 8)
                        rhs = kv1[:, :, :, wy, :, wx]
                        lhsT = qw[1][:, ci * 144 + q0:ci * 144 + q0 + nqsz]
                        nc.tensor.matmul(ps[:nqsz], lhsT, rhs,
                                         start=(ci == 0), stop=(ci == 63))
                    copy_alt(
                        s1_t[nqi][:nqsz, (f - 1) * 144:(f + 1) * 144],
                        ps[:nqsz])
        esA.close()

        # ============ phase B: softmax + P^T (both branches) ============
        esB = ExitStack()
        p_P = esB.enter_context(tc.tile_pool(name="P", bufs=1))
        p_stat = esB.enter_context(tc.tile_pool(name="stat", bufs=4))
        p0_t = [p_P.tile([128, 2304], BF16, name=f"p0_{i}", tag=f"p0_{i}")
                for i in range(len(NQB[0]))]
        p1_t = [p_P.tile([128, 576], BF16, name=f"p1_{i}", tag=f"p1_{i}")
                for i in range(len(NQB[1]))]

        for b in range(2):
            st = s0_t if b == 0 else s1_t
            pt = p0_t if b == 0 else p1_t
            for nqi, (q0, nqsz) in enumerate(NQB[b]):
                ls = p_stat.tile([128, 1], F32, name="ls", tag="ls")
                nc.scalar.activation(out=pt[nqi][:nqsz, :],
                                     in_=st[nqi][:nqsz, :], func=Exp,
                                     scale=SC[b], accum_out=ls[:nqsz])
                rs = p_stat.tile([128, 1], F32, name="rs", tag="rs")
                nc.vector.reciprocal(rs[:nqsz], ls[:nqsz])
                nc.vector.tensor_scalar_mul(pt[nqi][:nqsz, :],
                                            pt[nqi][:nqsz, :], rs[:nqsz])

        # transposes: P0 -> DRAM-spilled P0T; P1 -> resident P1T
        with tc.tile_pool(name="p0t", bufs=1) as p_P0T, \
             tc.tile_pool(name="ptps", bufs=4, space="PSUM") as p_ptps:
            for ti in range(18):
                p0t = p_P0T.tile([128, 576], BF16, name=f"p0t{ti % 3}",
                                 tag=f"p0t{ti % 3}", bufs=2)
                for nqi, (q0, nqsz) in enumerate(NQB[0]):
                    tp = p_ptps.tile([128, 128], BF16, name="ptps", tag="ptps")
                    nc.tensor.transpose(
                        tp[:, :nqsz],
                        p0_t[nqi][:nqsz, ti * 128:(ti + 1) * 128],
                        ident[:nqsz, :nqsz])
                    copy_alt(p0t[:, q0:q0 + nqsz], tp[:, :nqsz])
                nc.sync.dma_start(out=p0t_dram[:, ti * 576:(ti + 1) * 576],
                                  in_=p0t)
            for i, (f, r0, nr) in enumerate(MT1):
                mt = nr * 12
                c0 = f * 144 + r0 * 12
                for nqi, (q0, nqsz) in enumerate(NQB[1]):
                    tp = p_ptps.tile([128, 128], BF16, name="ptps", tag="ptps")
                    nc.tensor.transpose(
                        tp[:mt, :nqsz],
                        p1_t[nqi][:nqsz, c0:c0 + mt],
                        ident[:nqsz, :nqsz])
                    copy_alt(p1t_t[i][:mt, q0:q0 + nqsz], tp[:mt, :nqsz])
        esB.close()
        esS.close()

        # ============ phase C0+D0: V0^T build, then PV0 -> att0 ============
        esV0 = ExitStack()
        p_V0 = esV0.enter_context(tc.tile_pool(name="V0", bufs=1))
        v0_t = [p_V0.tile([128, 16 * 128], BF16, name=f"v0_{i}",
                          tag=f"v0_{i}") for i in range(18)]
        with ExitStack() as esC0:
            p_xw = esC0.enter_context(tc.tile_pool(name="xw0", bufs=2))
            p_x2 = esC0.enter_context(tc.tile_pool(name="c0x", bufs=2))
            p_vps = esC0.enter_context(
                tc.tile_pool(name="vps", bufs=6, space="PSUM"))
            for f in range(T):
                # window-major gather from quarter-frame x loads; xw0 is
                # double-buffered so frame f+1's gather overlaps f's matmuls
                xw0 = [p_xw.tile([128, 16 * 576], BF16, name=f"xw0_{cb}",
                                 tag=f"xw0_{cb}") for cb in range(2)]
                for q in range(4):
                    for cb in range(2):
                        xq = p_x2.tile([128, 2304], BF16, name=f"c0x{cb}",
                                       tag=f"c0x{cb}")
                        nc.sync.dma_start(
                            out=xq, in_=xv.ap()[f, cb * 128:(cb + 1) * 128,
                                                q * 2304:(q + 1) * 2304])
                        xqv = xq.rearrange(
                            "p (oh hh ow ww) -> p oh hh ow ww",
                            oh=6, hh=4, ow=24, ww=4)
                        for ci in range(16):
                            wy, wx = divmod(ci, 4)
                            dst = xw0[cb][:, ci * 576 + q * 144:
                                          ci * 576 + (q + 1) * 144].rearrange(
                                "p (a c) -> p a c", a=6)
                            copy_rr(dst, xqv[:, :, wy, :, wx])
                for (ti, off, m, j0) in _v0_pieces(f):
                    for cig in range(4):
                        ps = p_vps.tile([128, 512], F32, name="vps", tag="vps")
                        for cii in range(4):
                            ci = cig * 4 + cii
                            for cb in range(2):
                                nc.tensor.matmul(
                                    ps[off:off + m, cii * 128:(cii + 1) * 128],
                                    xw0[cb][:, ci * 576 + j0:ci * 576 + j0 + m],
                                    wv_sb[cb][:, 0:128],
                                    start=(cb == 0), stop=(cb == 1),
                                    tile_position=(0, off))
                        copy_alt(
                            v0_t[ti][off:off + m, cig * 512:(cig + 1) * 512],
                            ps[off:off + m, :])

        # D0: PV0 -> att0 -> DRAM
        with tc.tile_pool(name="att0", bufs=1) as p_att0, \
             tc.tile_pool(name="p0tr", bufs=1) as p_P0Tr, \
             tc.tile_pool(name="pvps", bufs=4, space="PSUM") as p_pvps:
            att0 = p_att0.tile([128, 98 * 98], BF16, name="att0", tag="att0")
            att0v = zero_borders(att0)
            wvw0 = att0v[:, 1:97, 1:97].rearrange(
                "p (oh hh) (ow ww) -> p oh hh ow ww", hh=4, ww=4)
            p0t_r = [p_P0Tr.tile([128, 576], BF16, name=f"p0tr{i}",
                                 tag=f"p0tr{i}") for i in range(18)]
            for ti in range(18):
                nc.sync.dma_start(out=p0t_r[ti],
                                  in_=p0t_dram[:, ti * 576:(ti + 1) * 576])
            for ci in range(16):
                wy, wx = divmod(ci, 4)
                for nqh in range(2):
                    ps = p_pvps.tile([128, 288], F32, name="pvps", tag="pvps")
                    for ti in range(18):
                        nc.tensor.matmul(
                            ps, v0_t[ti][:, ci * 128:(ci + 1) * 128],
                            p0t_r[ti][:, nqh * 288:(nqh + 1) * 288],
                            start=(ti == 0), stop=(ti == 17))
                    dst = wvw0[:, nqh * 12:(nqh + 1) * 12, wy, :, wx]
                    bias_copy_alt(dst, ps.rearrange("p (a c) -> p a c", a=12),
                                  bv_sb[:, 0:1])
            nc.sync.dma_start(out=att0_dram, in_=att0)
        esV0.close()

        # ==== phase C1+D1: V1^T via per-frame window gather, PV1 -> att1 ====
        esV1 = ExitStack()
        p_V1 = esV1.enter_context(tc.tile_pool(name="V1", bufs=1))
        v1_t = [p_V1.tile([nr * 12, 64 * 128], BF16,
                          name=f"v1_{i}", tag=f"v1_{i}")
                for i, (f, r0, nr) in enumerate(MT1)]
        with ExitStack() as esC1:
            p_xw1 = esC1.enter_context(tc.tile_pool(name="xw1", bufs=1))
            p_x3 = esC1.enter_context(tc.tile_pool(name="c1x", bufs=2))
            p_vps1 = esC1.enter_context(
                tc.tile_pool(name="vps1", bufs=6, space="PSUM"))
            xw1f = [p_xw1.tile([128, 64 * 144], BF16, name=f"xw1_{cb}",
                               tag=f"xw1_{cb}") for cb in range(2)]
            for f in range(T):
                for q in range(4):
                    for cb in range(2):
                        xq = p_x3.tile([128, 2304], BF16, name=f"c1x{cb}",
                                       tag=f"c1x{cb}")
                        nc.sync.dma_start(
                            out=xq, in_=xv.ap()[f, cb * 128:(cb + 1) * 128,
                                                q * 2304:(q + 1) * 2304])
                        xqv = xq.rearrange(
                            "p (oh hh ow ww) -> p oh hh ow ww",
                            oh=3, hh=8, ow=12, ww=8)
                        xwv = xw1f[cb].rearrange(
                            "p (ci a c) -> p ci a c", ci=64, a=12)
                        for wy in range(8):
                            dst = xwv[:, wy * 8:(wy + 1) * 8,
                                      q * 3:(q + 1) * 3, :]
                            src = xqv[:, :, wy, :, :].rearrange(
                                "p oh ow ww -> p ww oh ow")
                            copy_rr(dst, src)
                for i, (ff, r0, nr) in enumerate(MT1):
                    if ff != f:
                        continue
                    mt = nr * 12
                    for cig in range(16):
                        ps = p_vps1.tile([128, 512], F32,
                                         name="vps1", tag="vps1")
                        for cii in range(4):
                            ci = cig * 4 + cii
                            for cb in range(2):
                                nc.tensor.matmul(
                                    ps[0:mt, cii * 128:(cii + 1) * 128],
                                    xw1f[cb][:, ci * 144 + r0 * 12:
                                             ci * 144 + r0 * 12 + mt],
                                    wv_sb[cb][:, 128:256],
                                    start=(cb == 0), stop=(cb == 1))
                        copy_alt(
                            v1_t[i][0:mt, cig * 512:(cig + 1) * 512],
                            ps[0:mt, :])

        # att0 reload + 3x3 weights, hoisted so phase E can start during D1
        esE0 = ExitStack()
        p_attE = esE0.enter_context(tc.tile_pool(name="attE", bufs=1))
        p_wot = esE0.enter_context(tc.tile_pool(name="wot", bufs=1))
        att0E = p_attE.tile([128, 98 * 98], BF16, name="attE0", tag="attE0")
        nc.sync.dma_start(out=att0E, in_=att0_dram)
        wot_sb = []
        for cb in range(2):
            t = p_wot.tile([128, 9, C], BF16, name=f"wot{cb}", tag=f"wot{cb}")
            nc.sync.dma_start(
                out=t,
                in_=wot.ap()[:, cb * 128:(cb + 1) * 128, :].rearrange(
                    "t i o -> i t o"))
            wot_sb.append(t)

        # D1: PV1 -> att1 (stays resident through E)
        esAtt1 = ExitStack()
        p_att1 = esAtt1.enter_context(tc.tile_pool(name="att1", bufs=1))
        att1 = p_att1.tile([128, 98 * 98], BF16, name="att1", tag="att1")
        att1v = zero_borders(att1)
        wvw1 = att1v[:, 1:97, 1:97].rearrange(
            "p (oh hh) (ow ww) -> p oh hh ow ww", hh=8, ww=8)
        with tc.tile_pool(name="pvps1", bufs=4, space="PSUM") as p_pvps1:
            for ci in range(64):
                wy, wx = divmod(ci, 8)
                ps = p_pvps1.tile([128, 144], F32, name="pvps1", tag="pvps1")
                for i, (f, r0, nr) in enumerate(MT1):
                    mt = nr * 12
                    nc.tensor.matmul(
                        ps, v1_t[i][:mt, ci * 128:(ci + 1) * 128],
                        p1t_t[i][:mt, :],
                        start=(i == 0), stop=(i == len(MT1) - 1))
                dst = wvw1[:, :, wy, :, wx]
                bias_copy_alt(dst, ps.rearrange("p (a c) -> p a c", a=12),
                              bv_sb[:, 1:2])

        # ============ phase E: 3x3 conv + LeakyReLU ============
        # att1/att0E/wot stay resident (pools close at the very end)
        with tc.tile_pool(name="eout", bufs=2) as p_eo, \
             tc.tile_pool(name="dps", bufs=8, space="PSUM") as p_dps:
            att = [att0E, att1]
            attv2 = [att[cb].rearrange("p (h w) -> p h w", h=98)
                     for cb in range(2)]
            for coutb in range(2):
                for g in range(4):          # groups of 6 rg (24 rows each)
                    pst = [p_dps.tile([128, 384], F32, name="dps",
                                      tag="dps") for _ in range(6)]
                    ot = p_eo.tile([128, 6 * 384], F32, name="eo", tag="eo")
                    k = 0
                    for cb in range(2):
                        for tap in range(9):
                            dy, dx = divmod(tap, 3)
                            lhsT = wot_sb[cb][:, tap,
                                              coutb * 128:(coutb + 1) * 128]
                            for ri in range(6):
                                rg = g * 6 + ri
                                rhs = attv2[cb][:, rg * 4 + dy:rg * 4 + dy + 4,
                                                dx:dx + 96]
                                nc.tensor.matmul(pst[ri], lhsT, rhs,
                                                 start=(k == 0), stop=(k == 17))
                            k += 1
                    for ri in range(6):
                        t1 = p_eo.tile([128, 384], F32, name="t1", tag="t1",
                                       bufs=3)
                        nc.scalar.activation(
                            out=t1, in_=pst[ri], func=Identity,
                            bias=bo_sb[:, coutb:coutb + 1], scale=1.0)
                        nc.vector.scalar_tensor_tensor(
                            out=ot[:, ri * 384:(ri + 1) * 384], in0=t1,
                            scalar=0.2, in1=t1,
                            op0=mybir.AluOpType.mult,
                            op1=mybir.AluOpType.max)
                    nc.sync.dma_start(
                        out=out.ap()[coutb * 128:(coutb + 1) * 128,
                                     g * 2304:(g + 1) * 2304],
                        in_=ot)
        esAtt1.close()
        esE0.close()
        esV1.close()
        esPT1.close()
    return nc


_CACHED = {}


def _get_nc():
    if "nc" not in _CACHED:
        nc = bacc.Bacc("TRN2", debug=False, target_bir_lowering=False)
        build(nc)
        nc.compile()
        _CACHED["nc"] = nc
    return _CACHED["nc"]


def make_in_maps(x, wq, bq_, wk, bk_, wv, bv_, wo, bo_):
    import ml_dtypes
    bf16 = ml_dtypes.bfloat16
    shared = {
        "wqt": np.ascontiguousarray(wq.T).astype(bf16),
        "wkt": np.ascontiguousarray(wk.T).astype(bf16),
        "wvt": np.ascontiguousarray(wv.T).astype(bf16),
        "wot": np.ascontiguousarray(
            wo.transpose(2, 3, 1, 0).reshape(9, C, C)).astype(bf16),
        "bq": np.ascontiguousarray(bq_.astype(np.float32)),
        "bk": np.ascontiguousarray(bk_.astype(np.float32)),
        "bv": np.ascontiguousarray(bv_.astype(np.float32)),
        "bo": np.ascontiguousarray(bo_.astype(np.float32)),
    }
    x3 = x.reshape(2 * T, C, PIX).astype(bf16)
    in_maps = []
    for core in range(NCORES):
        v, f = divmod(core, T)
        idx = [v * T + (f + i) % T for i in range(T)]
        m = dict(shared)
        m["xv"] = np.ascontiguousarray(x3[idx])
        in_maps.append(m)
    return in_maps


def kernel(**inputs):
    from concourse.bass_utils import run_bass_kernel_spmd

    x = np.asarray(inputs["x"], dtype=np.float32)
    in_maps = make_in_maps(
        x, np.asarray(inputs["wq"]), np.asarray(inputs["bq"]),
        np.asarray(inputs["wk"]), np.asarray(inputs["bk"]),
        np.asarray(inputs["wv"]), np.asarray(inputs["bv"]),
        np.asarray(inputs["wo"]), np.asarray(inputs["bo"]))
    nc = _get_nc()
    res = run_bass_kernel_spmd(nc, in_maps, core_ids=list(range(NCORES)))
    outs = [res.results[c]["out"].reshape(C, H, W) for c in range(NCORES)]
    return np.stack(outs).astype(np.float32)
